# revision 37
# baseline (speedup 1.0000x reference)
"""Trainium2 Bass kernel for nn_MultiHeadAttention (B=2, S=2048, E=1024, H=16, d=64).

Sharding: 8 cores = 2 batches x 4 head-groups (4 heads each).
Per core: QKV projection (transposed layout), causal flash-style attention
(transposed softmax, no max subtraction), o_proj partial; host sums the
4 partials per batch (the tensor-parallel all-reduce, done at gather time).

All matmuls in bf16 with fp32 PSUM accumulation.
"""
import sys

sys.path.insert(0, "/opt/trn_rl_repo")

import math

import ml_dtypes
import numpy as np

import concourse.bacc as bacc_mod
import concourse.tile as tile
from concourse import mybir
from concourse.bass_utils import run_bass_kernel_spmd

F32 = mybir.dt.float32
BF16 = mybir.dt.bfloat16
FP8 = mybir.dt.float8e4
AF = mybir.ActivationFunctionType
ALU = mybir.AluOpType
DR = mybir.MatmulPerfMode.DoubleRow

B, S, E = 2, 2048, 1024
H, D = 16, 64           # total heads, head dim
HG = 4                  # heads per core (group)
NC_ = 8                 # cores
SC = S // 128           # 16 sequence chunks of 128
INV_SQRT_D = 1.0 / math.sqrt(D)

BF = ml_dtypes.bfloat16
F8 = ml_dtypes.float8_e4m3


def build_nc():
    nc = bacc_mod.Bacc(target_bir_lowering=False)

    # ---- DRAM tensors (per-core shards, prepared on host) ----
    # fp8 DoubleRow compensated QKV, uniformly 16x scaled:
    #   16*qkv = (16*W8)@X8 + (16*W8)@Xr8 + Wr16@X8
    # with Wr16 = q8(16*(W - W8)).  The 16x cancels via exp-scale/256 (Q,K)
    # and a 16.0 ones-column in vh (V numerator and denominator both 16x).
    # Paired tensors are packed (v-dim: 0=main/X8, 1=residual) so each strip
    # is a single HWDGE issue.
    xx_d = nc.dram_tensor("xx", [4, E, 2, 512], FP8, kind="ExternalInput")
    wq_d = nc.dram_tensor("wq", [E, 2, 2 * HG * D], FP8, kind="ExternalInput")
    wvv_d = nc.dram_tensor("wvv", [E, 2, HG * D], FP8, kind="ExternalInput")
    wo_d = nc.dram_tensor("wo", [128, 2, E], BF16, kind="ExternalInput")
    msk_d = nc.dram_tensor("msk", [128, 2, 128], BF16, kind="ExternalInput")
    bias_d = nc.dram_tensor("bias", [128, 4 + HG * D], F32, kind="ExternalInput")
    y_d = nc.dram_tensor("y", [S, E], BF16, kind="ExternalOutput")

    with tile.TileContext(nc) as tc:
        import contextlib
        with contextlib.ExitStack() as ctx:
            persist = ctx.enter_context(tc.tile_pool(name="persist", bufs=1))
            dve_tmp = ctx.enter_context(tc.tile_pool(name="dve_tmp", bufs=4))
            expt_pool = ctx.enter_context(tc.tile_pool(name="expt_pool", bufs=7))

            # ---- persistent SBUF tiles ----
            # X.T [e-part, strip, e-chunk, v, s%512]: strip-major so each
            # strip loads as one contiguous DMA
            xx = persist.tile([128, 4, 8, 2, 512], FP8)
            wq = persist.tile([128, 8, 2, 2 * HG * D], FP8)
            wvv = persist.tile([128, 8, 2, HG * D], FP8)
            wo = persist.tile([128, 2, E], BF16)
            msk = persist.tile([128, 2, 128], BF16)  # 0=ident, 1=tneg
            bias = persist.tile([128, 4 + HG * D], F32)
            qt = persist.tile([128, 2, S], BF16)            # Q.T
            kt = persist.tile([128, 2, S], BF16)            # K.T
            # V-hat: col 0 = ones (denominator row), cols 1-63 zero pad
            # (engine partition APs must fit aligned power-of-2 blocks),
            # cols 64-127 = V
            vh = persist.tile([128, SC, HG, 128], BF16)
            outt = persist.tile([128, 2, S], BF16)          # even-head staging (rows 64-127)
            outt2 = persist.tile([128, 2, S], BF16)         # stacked for o_proj

            xx_dr = xx_d.ap().rearrange("t (o p) v s -> p t o v s", p=128)
            wq_dr = wq_d.ap().rearrange("(o p) v f -> p o v f", p=128)
            # strip-0 loads interleaved in the order phase A consumes them,
            # chunked fine at the front so the first matmul starts early
            for e0, e1 in ((0, 1), (1, 2), (2, 4), (4, 8)):
                nc.sync.dma_start(wq[:, e0:e1], wq_dr[:, e0:e1])
                nc.sync.dma_start(xx[:, 0, e0:e1], xx_dr[:, 0, e0:e1])
            nc.sync.dma_start(
                wvv[:], wvv_d.ap().rearrange("(o p) v f -> p o v f", p=128))
            nc.sync.dma_start(msk[:], msk_d.ap())
            nc.sync.dma_start(bias[:], bias_d.ap())
            # vh cols 1-63 stay uninitialized: they only feed PSUM rows 1-63,
            # which are never read (row 0 = denom, rows 64-127 = V out).
            # 16.0 matches the uniform 16x scale of the QKV psum (see above).
            nc.vector.memset(vh[:, :, :, 0], 16.0)

            ab_ctx = ctx.enter_context(contextlib.ExitStack())
            ps_a = ab_ctx.enter_context(tc.tile_pool(name="ps_a", bufs=2, space="PSUM"))
            ps_sc = ab_ctx.enter_context(tc.tile_pool(name="ps_sc", bufs=2, space="PSUM"))
            ps_pv = ab_ctx.enter_context(tc.tile_pool(name="ps_pv", bufs=2, space="PSUM"))

            # ================= Phase A: QKV projection =================
            # emitted per 512-wide strip; strips 2,3 are interleaved into
            # attention pass 0 (which only needs strips 0,1) as PE filler.
            # compensated-fp8 accumulation: 3 terms x 4 e-pairs, DoubleRow
            # (2 k-tiles per call at 0.5 cyc/col -> 25% fewer PE cycles than
            # bf16 with full-precision-class accuracy).  Terms index the
            # packed v-dim: (w, x) in {(main, X8), (main, Xr), (resid, X8)}.
            A_TERMS = [(0, 0), (0, 1), (1, 0)]

            def emit_a_strip(s4, parts=(0, 1)):
                sl = slice(512 * s4, 512 * (s4 + 1))
                if s4 > 0 and 0 in parts:
                    nc.sync.dma_start(xx[:, s4], xx_dr[:, s4])
                if 0 not in parts:
                    for_range = ()
                else:
                    for_range = range(4)
                for f in for_range:                         # q0 q1 k0 k1
                    ps = ps_a.tile([128, 512], F32, tag="mm", name=f"qk_{s4}_{f}")
                    # e-pair-major: consume each arriving chunk fully before
                    # needing the next (wq packs main+residual per chunk)
                    for e2 in range(4):
                        for i, (wv_, xv_) in enumerate(A_TERMS):
                            nc.tensor.matmul(
                                ps[:],
                                wq[:, 2 * e2:2 * e2 + 2, wv_, 128 * f:128 * (f + 1)],
                                xx[:, s4, 2 * e2:2 * e2 + 2, xv_, :],
                                start=(i == 0 and e2 == 0),
                                stop=(i == 2 and e2 == 3), perf_mode=DR)
                    dst = (qt if f < 2 else kt)[:, f % 2, sl]
                    nc.vector.tensor_tensor(
                        dst, ps[:], bias[:, f:f + 1].to_broadcast([128, 512]), ALU.add)
                if 1 not in parts:
                    return
                for ss in range(4):                         # V: 128-row blocks
                    s = 4 * s4 + ss
                    ps = ps_a.tile([128, 512], F32, tag="mm", name=f"v_{s4}_{ss}")
                    psv = ps[:, :HG * D]
                    for e2 in range(4):
                        for i, (wv_, xv_) in enumerate(A_TERMS):
                            nc.tensor.matmul(
                                psv,
                                xx[:, s4, 2 * e2:2 * e2 + 2, xv_, 128 * ss:128 * (ss + 1)],
                                wvv[:, 2 * e2:2 * e2 + 2, wv_, :],
                                start=(i == 0 and e2 == 0),
                                stop=(i == 2 and e2 == 3), perf_mode=DR)
                    nc.vector.tensor_tensor(
                        vh[:, s, :, 64:128],
                        psv.rearrange("p (h c) -> p h c", h=HG),
                        bias[:, 4:].rearrange("p (h c) -> p h c", h=HG),
                        ALU.add)

            emit_a_strip(0)
            emit_a_strip(1)
            # wo is first needed at P1 h=1: issue after the strip-1 loads so
            # it doesn't delay the phase-A stream
            nc.sync.dma_start(wo[:], wo_d.ap())

            # ================= Phase B: attention, pass-major ================
            # Two passes over the key chunks j per head: pass 0 accumulates
            # query columns [0, 1024) (t=0,1), pass 1 columns [1024, 2048)
            # (t=2,3).  Each (j, column) score is computed exactly once; this
            # halves PV PSUM residency so phase A pools stay open and QKV
            # matmuls fill PE stalls during the exp-bound stretches.  o_proj
            # for each column half is emitted right after the half completes
            # (its PSUM tiles share the "sc" slots), overlapping with the
            # next pass / other heads.
            out_sb = ctx.enter_context(tc.tile_pool(name="out_sb", bufs=8))

            def emit_oproj(s_list, pool, tag, eng=("dve", "dve"), split_dma=False):
                for s in s_list:
                    o = out_sb.tile([128, E], BF16, tag="o")
                    for eh in range(2):
                        ps = pool.tile([128, 512], F32, tag=tag,
                                       name=f"oproj_{s}_{eh}")
                        for c in range(2):
                            nc.tensor.matmul(
                                ps[:], outt2[:, c, 128 * s:128 * (s + 1)],
                                wo[:, c, 512 * eh:512 * (eh + 1)],
                                start=(c == 0), stop=(c == 1))
                        if eng[eh] == "dve":
                            nc.vector.tensor_copy(
                                o[:, 512 * eh:512 * (eh + 1)], ps[:])
                        else:
                            nc.scalar.copy(o[:, 512 * eh:512 * (eh + 1)], ps[:])
                        if split_dma:
                            # tail mode: ship each half as soon as its copy
                            # lands to shorten the final drain
                            nc.sync.dma_start(
                                y_d.ap()[128 * s:128 * (s + 1),
                                         512 * eh:512 * (eh + 1)],
                                o[:, 512 * eh:512 * (eh + 1)])
                    if not split_dma:
                        # one full-row DMA per s-chunk: halves HWDGE issues
                        nc.sync.dma_start(y_d.ap()[128 * s:128 * (s + 1), :], o[:])

            for P_ in range(2):
                cl, ch = 1024 * P_, 1024 * (P_ + 1)         # column range
                jmax = 8 if P_ == 0 else 16
                for h in range(HG):
                    hk, hp = h // 2, 64 * (h % 2)
                    kts = kt[hp:hp + 64, hk, :]
                    qts = qt[hp:hp + 64, hk, :]
                    pv_tiles = {
                        t: ps_pv.tile([128, 512], F32, tag="pv",
                                      name=f"pv_{h}_{t}")
                        for t in (2 * P_, 2 * P_ + 1)}
                    # the two narrowest j rows of each pass share one psum
                    # tile/exp op (right-aligned, abutting regions): fewer
                    # ACT ops and shorter tail chains per (head, pass) unit
                    if P_ == 0:
                        groups = [(0,), (1,), (2,), (3,), (5, 4), (7, 6)]
                    else:
                        # stop-order constraints: j11 (t=2 stop) after j10,
                        # j15 (t=3 stop) in the last group
                        groups = ([(j,) for j in range(11)]
                                  + [(13, 11), (15, 14, 12)])
                    for grp in groups:
                        # per-member tile column offset (singles: a0-cl;
                        # pairs: packed right-aligned into [B1, 1024))
                        offs = {}
                        pos = 1024
                        for j in sorted(grp, reverse=True):   # wide first
                            a0 = max(128 * j, cl)
                            pos -= ch - a0
                            offs[j] = pos
                        expt = expt_pool.tile([128, 1024], BF16, tag="expt",
                                              name=f"expt_{h}_{P_}_{grp[0]}")
                        sc_ps = ps_sc.tile([128, 1024], F32, tag="sc")
                        # qt/kt carry a 16x scale each -> exp scale /256
                        for j in grp:
                            lo = 128 * j
                            a0 = max(lo, cl)
                            off = offs[j]
                            for a in range(a0 - a0 % 512, ch, 512):
                                aa = max(a, a0)
                                diag = (aa == lo)       # seg with the diagonal
                                nc.tensor.matmul(
                                    sc_ps[:, off + aa - a0:off + a + 512 - a0],
                                    kts[:, lo:lo + 128], qts[:, aa:a + 512],
                                    start=True, stop=not diag)
                                if diag:
                                    # causal mask: add -1e30 at cols q' < k of
                                    # the 128x128 diagonal tile via I.T @ tneg
                                    nc.tensor.matmul(
                                        sc_ps[:, off:off + 128],
                                        msk[:, 0, :], msk[:, 1, :], start=False,
                                        stop=True)
                        gmin = min(offs.values())
                        if gmin == 0:
                            # full-width group: split the exp at the t
                            # boundary so PV of the low half starts earlier
                            # (and the two halves pipeline on ACT)
                            nc.scalar.activation(
                                expt[:, 0:512], sc_ps[:, 0:512],
                                AF.Exp, scale=INV_SQRT_D / 256.0)
                            nc.scalar.activation(
                                expt[:, 512:1024], sc_ps[:, 512:1024],
                                AF.Exp, scale=INV_SQRT_D / 256.0)
                        else:
                            nc.scalar.activation(
                                expt[:, gmin:], sc_ps[:, gmin:],
                                AF.Exp, scale=INV_SQRT_D / 256.0)
                        # PV accumulation (+ denominator row 0 via ones column)
                        for j in sorted(grp):
                            lo = 128 * j
                            a0 = max(lo, cl)
                            off = offs[j]
                            for t in (2 * P_, 2 * P_ + 1):
                                a = max(512 * t, lo)
                                if a >= 512 * (t + 1):
                                    continue
                                nc.tensor.matmul(
                                    pv_tiles[t][:, a - 512 * t:512],
                                    vh[:, j, h, :],
                                    expt[:, off + a - a0:off + 512 * (t + 1) - a0],
                                    start=(j == 0), stop=(j == 4 * t + 3))
                                if j == 4 * t + 3:
                                    # normalize once this sq-chunk completes
                                    rec = dve_tmp.tile([1, 512], F32, tag="rec",
                                                       name=f"rec_{h}_{t}")
                                    bc = dve_tmp.tile([128, 512], F32, tag="bc",
                                                      name=f"bc_{h}_{t}")
                                    nc.vector.reciprocal(rec[:],
                                                         pv_tiles[t][0:1, :])
                                    nc.gpsimd.partition_broadcast(bc[:], rec[:])
                                    tsl = slice(512 * t, 512 * (t + 1))
                                    if h % 2 == 1:
                                        # odd heads: partitions already match
                                        # outt2's upper half — write direct
                                        nc.vector.tensor_tensor(
                                            outt2[64:128, h // 2, tsl],
                                            pv_tiles[t][64:128, :],
                                            bc[64:128, :], ALU.mult)
                                    else:
                                        nc.vector.tensor_tensor(
                                            outt[64:128, h // 2, tsl],
                                            pv_tiles[t][64:128, :],
                                            bc[64:128, :], ALU.mult)
                                        # cross-partition stack to rows 0-63
                                        nc.sync.dma_start(
                                            outt2[0:64, h // 2, tsl],
                                            outt[64:128, h // 2, tsl])
                        if P_ == 1 and h == 3 and 11 in grp:
                            # all heads' t=2 columns are final: overlap their
                            # o_proj with the last head's t=3 tail
                            emit_oproj(range(8, 10), ps_a, "mm")

                    if P_ == 0:
                        # PE filler during the exp-bound stretches
                        [lambda: emit_a_strip(2, (0,)),
                         lambda: emit_a_strip(2, (1,)),
                         lambda: emit_a_strip(3, (0,)),
                         lambda: emit_a_strip(3, (1,))][h]()
                    if P_ == 1 and h == 1:
                        emit_oproj(range(0, 8), ps_a, "mm")  # low-priority filler

            # s=10..15 run after the A/B pools close, with a deeper dedicated
            # PSUM pool for a tighter copy/DMA pipeline (s=8..9 were
            # interleaved into head 3's pass-1 stretch above).
            ab_ctx.close()
            with tc.tile_pool(name="ps_c", bufs=6, space="PSUM") as ps_c:
                emit_oproj(range(10, 16), ps_c, "oproj", eng=("act", "dve"),
                           split_dma=True)
    nc.compile()
    return nc


_NC_CACHE = {}


def _get_nc():
    if "nc" not in _NC_CACHE:
        _NC_CACHE["nc"] = build_nc()
    return _NC_CACHE["nc"]


def kernel(X, mask, W_qkv, b_qkv, W_o, b_o):
    X = np.asarray(X, dtype=np.float32)
    W_qkv = np.asarray(W_qkv, dtype=np.float32)
    b_qkv = np.asarray(b_qkv, dtype=np.float32)
    W_o = np.asarray(W_o, dtype=np.float32)
    b_o = np.asarray(b_o, dtype=np.float32)

    ident = np.eye(128, dtype=np.float32).astype(BF)
    r = np.arange(128)
    tneg = np.where(r[None, :] < r[:, None], np.float32(-1e30), np.float32(0.0))
    tneg = tneg.astype(BF)                      # tneg[p, n] = -1e30 if n < p

    # fp8 compensation operands (dtype prep for the device kernel):
    # X ~ X8 + Xr8;  W ~ W8 + Wr16/16.  Device computes the uniformly
    # 16x-scaled 16*qkv = (16W8)@X8 + (16W8)@Xr8 + Wr16@X8, so biases ship
    # 16x and the vh ones-column is 16.0.  Pairs pack along a v-dim so each
    # strip/tensor is one DMA issue.
    xv = []
    for b in range(B):
        xT = np.ascontiguousarray(X[b].T)
        x8 = xT.astype(F8)
        xr = (xT - x8.astype(np.float32)).astype(F8)
        # [4, E, 2, 512]: strip-major, (X8, Xr) packed per strip
        xs = np.stack([x8.reshape(E, 4, 512), xr.reshape(E, 4, 512)], axis=2)
        xv.append(np.ascontiguousarray(xs.transpose(1, 0, 2, 3)))
    W8f = W_qkv.astype(F8).astype(np.float32)
    W16 = (16.0 * W8f).astype(F8)              # exact exponent shift
    Wr16 = (16.0 * (W_qkv - W8f)).astype(F8)
    msk = np.stack([ident, tneg], axis=1)      # [128, 2, 128]

    in_maps = []
    for c in range(NC_):
        b, g = c // 4, c % 4
        cols = slice(256 * g, 256 * (g + 1))
        kcols = slice(1024 + 256 * g, 1024 + 256 * (g + 1))
        vcols = slice(2048 + 256 * g, 2048 + 256 * (g + 1))
        wq = np.stack([
            np.concatenate([W16[:, cols], W16[:, kcols]], axis=1),
            np.concatenate([Wr16[:, cols], Wr16[:, kcols]], axis=1)], axis=1)
        wvv = np.stack([W16[:, vcols], Wr16[:, vcols]], axis=1)
        wo = np.ascontiguousarray(
            W_o[256 * g:256 * (g + 1), :].reshape(2, 128, E).transpose(1, 0, 2)).astype(BF)
        bqk = 16.0 * np.concatenate(
            [b_qkv[cols], b_qkv[kcols]]).reshape(4, 128).T.astype(np.float32)
        bv = 16.0 * np.broadcast_to(b_qkv[vcols], (128, 256)).astype(np.float32)
        bias = np.concatenate([bqk, bv], axis=1).astype(np.float32)
        in_maps.append({"xx": xv[b], "wq": np.ascontiguousarray(wq),
                        "wvv": np.ascontiguousarray(wvv), "wo": wo,
                        "msk": np.ascontiguousarray(msk),
                        "bias": np.ascontiguousarray(bias)})

    nc = _get_nc()
    res = run_bass_kernel_spmd(nc, in_maps, core_ids=list(range(NC_)))

    Y = np.zeros((B, S, E), dtype=np.float32)
    for c in range(NC_):
        Y[c // 4] += res.results[c]["y"].astype(np.float32)
    Y += b_o[None, None, :]
    return Y



# revision 53
# speedup vs baseline: 1.1041x; 1.1041x over previous
"""Trainium2 Bass kernel for nn_MultiHeadAttention (B=2, S=2048, E=1024, H=16, d=64).

Sharding: 8 cores = 2 batches x 4 head-groups (4 heads each).
Per core: QKV projection (transposed layout), causal flash-style attention
(transposed softmax, no max subtraction), o_proj partial; host sums the
4 partials per batch (the tensor-parallel all-reduce, done at gather time).

All matmuls in bf16 with fp32 PSUM accumulation.
"""
import sys

sys.path.insert(0, "/opt/trn_rl_repo")

import math

import ml_dtypes
import numpy as np

import concourse.bacc as bacc_mod
import concourse.tile as tile
from concourse import mybir
from concourse.bass_utils import run_bass_kernel_spmd

F32 = mybir.dt.float32
BF16 = mybir.dt.bfloat16
FP8 = mybir.dt.float8e4
AF = mybir.ActivationFunctionType
ALU = mybir.AluOpType
DR = mybir.MatmulPerfMode.DoubleRow

B, S, E = 2, 2048, 1024
H, D = 16, 64           # total heads, head dim
HG = 4                  # heads per core (group)
NC_ = 8                 # cores
SC = S // 128           # 16 sequence chunks of 128
INV_SQRT_D = 1.0 / math.sqrt(D)

BF = ml_dtypes.bfloat16
F8 = ml_dtypes.float8_e4m3


def build_nc():
    nc = bacc_mod.Bacc(target_bir_lowering=False)

    # ---- DRAM tensors (per-core shards, prepared on host) ----
    # fp8 DoubleRow compensated QKV, uniformly 16x scaled:
    #   16*qkv = (16*W8)@X8 + (16*W8)@Xr8 + Wr16@X8
    # with Wr16 = q8(16*(W - W8)).  The 16x cancels via exp-scale/256 (Q,K)
    # and a 16.0 ones-column in vh (V numerator and denominator both 16x).
    # Paired tensors are packed (v-dim: 0=main/X8, 1=residual) so each strip
    # is a single HWDGE issue.
    xx_d = nc.dram_tensor("xx", [4, E, 2, 512], FP8, kind="ExternalInput")
    wq_d = nc.dram_tensor("wq", [E, 2, 2 * HG * D], FP8, kind="ExternalInput")
    wvv_d = nc.dram_tensor("wvv", [E, 2, HG * D], FP8, kind="ExternalInput")
    wo_d = nc.dram_tensor("wo", [128, 2, E], BF16, kind="ExternalInput")
    msk_d = nc.dram_tensor("msk", [128, 2, 128], BF16, kind="ExternalInput")
    bias_d = nc.dram_tensor("bias", [128, 4 + HG * D], F32, kind="ExternalInput")
    y_d = nc.dram_tensor("y", [S, E], BF16, kind="ExternalOutput")

    with tile.TileContext(nc) as tc:
        import contextlib
        with contextlib.ExitStack() as ctx:
            persist = ctx.enter_context(tc.tile_pool(name="persist", bufs=1))
            dve_tmp = ctx.enter_context(tc.tile_pool(name="dve_tmp", bufs=4))
            expt_pool = ctx.enter_context(tc.tile_pool(name="expt_pool", bufs=10))

            # ---- persistent SBUF tiles ----
            # X.T [e-part, strip, e-chunk, v, s%512]: strip-major so each
            # strip loads as one contiguous DMA
            xx = persist.tile([128, 4, 8, 2, 512], FP8)
            wq = persist.tile([128, 8, 2, 2 * HG * D], FP8)
            wvv = persist.tile([128, 8, 2, HG * D], FP8)
            wo = persist.tile([128, 2, E], BF16)
            msk = persist.tile([128, 2, 128], BF16)  # 0=ident, 1=tneg
            bias = persist.tile([128, 4 + HG * D], F32)
            qt = persist.tile([128, 2, S], BF16)            # Q.T
            kt = persist.tile([128, 2, S], BF16)            # K.T
            # V-hat: col 0 = 16s (denominator row), cols 1-63 uninitialized
            # (they only feed PSUM rows 1-63, which are never read),
            # cols 64-127 = V
            vh = persist.tile([128, SC, HG, 128], BF16)
            outt = persist.tile([128, 2, S], BF16)          # even-head staging
            outt2 = persist.tile([128, 2, S], BF16)         # o_proj stationary

            xx_dr = xx_d.ap().rearrange("t (o p) v s -> p t o v s", p=128)
            wq_dr = wq_d.ap().rearrange("(o p) v f -> p o v f", p=128)
            # strip-0 loads interleaved in the order phase A consumes them,
            # chunked fine at the front so the first matmul starts early
            for e0, e1 in ((0, 2), (2, 4), (4, 8)):
                nc.sync.dma_start(wq[:, e0:e1], wq_dr[:, e0:e1])
                nc.sync.dma_start(xx[:, 0, e0:e1], xx_dr[:, 0, e0:e1])
            nc.sync.dma_start(bias[:], bias_d.ap())
            nc.sync.dma_start(
                wvv[:], wvv_d.ap().rearrange("(o p) v f -> p o v f", p=128))
            # strip 1 hoisted before the mask constants: phase A consumes it
            # ~8us in while msk isn't needed until the first diagonal block
            nc.sync.dma_start(xx[:, 1], xx_dr[:, 1])
            nc.sync.dma_start(msk[:], msk_d.ap())
            # 16.0 denominator column matches the uniform 16x QKV psum scale
            nc.vector.memset(vh[:, :, :, 0], 16.0)

            ab_ctx = ctx.enter_context(contextlib.ExitStack())
            ps_a = ab_ctx.enter_context(tc.tile_pool(name="ps_a", bufs=2, space="PSUM"))
            ps_sc = ab_ctx.enter_context(tc.tile_pool(name="ps_sc", bufs=4, space="PSUM"))
            ps_pv = ab_ctx.enter_context(tc.tile_pool(name="ps_pv", bufs=2, space="PSUM"))

            # ================= Phase A: QKV projection =================
            # emitted per 512-wide strip; strips 2,3 are interleaved into
            # attention pass 0 (which only needs strips 0,1) as PE filler.
            # compensated-fp8 accumulation: 3 terms x 4 e-pairs, DoubleRow
            # (2 k-tiles per call at 0.5 cyc/col -> 25% fewer PE cycles than
            # bf16 with full-precision-class accuracy).  Terms index the
            # packed v-dim: (w, x) in {(main, X8), (main, Xr), (resid, X8)}.
            A_TERMS = [(0, 0), (0, 1), (1, 0)]

            def emit_a_strip(s4, parts=(0, 1)):
                sl = slice(512 * s4, 512 * (s4 + 1))
                if s4 > 1 and 0 in parts:       # strip 1 preloaded above
                    nc.sync.dma_start(xx[:, s4], xx_dr[:, s4])
                if 0 not in parts:
                    for_range = ()
                else:
                    for_range = range(4)
                for f in for_range:                         # q0 q1 k0 k1
                    ps = ps_a.tile([128, 512], F32, tag="mm", name=f"qk_{s4}_{f}")
                    # e-pair-major: consume each arriving chunk fully before
                    # needing the next (wq packs main+residual per chunk)
                    for e2 in range(4):
                        for i, (wv_, xv_) in enumerate(A_TERMS):
                            nc.tensor.matmul(
                                ps[:],
                                wq[:, 2 * e2:2 * e2 + 2, wv_, 128 * f:128 * (f + 1)],
                                xx[:, s4, 2 * e2:2 * e2 + 2, xv_, :],
                                start=(i == 0 and e2 == 0),
                                stop=(i == 2 and e2 == 3), perf_mode=DR)
                    dst = (qt if f < 2 else kt)[:, f % 2, sl]
                    nc.vector.tensor_tensor(
                        dst, ps[:], bias[:, f:f + 1].to_broadcast([128, 512]), ALU.add)
                if 1 not in parts:
                    return
                for ss in range(4):                         # V: 128-row blocks
                    s = 4 * s4 + ss
                    ps = ps_a.tile([128, 512], F32, tag="mm", name=f"v_{s4}_{ss}")
                    psv = ps[:, :HG * D]
                    for e2 in range(4):
                        for i, (wv_, xv_) in enumerate(A_TERMS):
                            nc.tensor.matmul(
                                psv,
                                xx[:, s4, 2 * e2:2 * e2 + 2, xv_, 128 * ss:128 * (ss + 1)],
                                wvv[:, 2 * e2:2 * e2 + 2, wv_, :],
                                start=(i == 0 and e2 == 0),
                                stop=(i == 2 and e2 == 3), perf_mode=DR)
                    nc.vector.tensor_tensor(
                        vh[:, s, :, 64:128],
                        psv.rearrange("p (h c) -> p h c", h=HG),
                        bias[:, 4:].rearrange("p (h c) -> p h c", h=HG),
                        ALU.add)

            emit_a_strip(0)
            emit_a_strip(1)
            # wo is first needed at P1 h=1: issue after the strip-1 loads so
            # it doesn't delay the phase-A stream
            nc.sync.dma_start(wo[:], wo_d.ap())

            # ================= Phase B: attention, pass-major ================
            # Two passes over the key chunks j per head: pass 0 accumulates
            # query columns [0, 1024) (t=0,1), pass 1 columns [1024, 2048)
            # (t=2,3).  Each (j, column) score is computed exactly once; this
            # halves PV PSUM residency so phase A pools stay open and QKV
            # matmuls fill PE stalls during the exp-bound stretches.  o_proj
            # for each column half is emitted right after the half completes
            # (its PSUM tiles share the "sc" slots), overlapping with the
            # next pass / other heads.
            out_sb = ctx.enter_context(tc.tile_pool(name="out_sb", bufs=8))

            def emit_oproj(s_list, pool, tag, eng=("dve", "dve"), split_dma=False):
                for s in s_list:
                    o = out_sb.tile([128, E], BF16, tag="o")
                    for eh in range(2):
                        ps = pool.tile([128, 512], F32, tag=tag,
                                       name=f"oproj_{s}_{eh}")
                        for c in range(2):
                            nc.tensor.matmul(
                                ps[:], outt2[:, c, 128 * s:128 * (s + 1)],
                                wo[:, c, 512 * eh:512 * (eh + 1)],
                                start=(c == 0), stop=(c == 1))
                        if eng[eh] == "dve":
                            nc.vector.tensor_copy(
                                o[:, 512 * eh:512 * (eh + 1)], ps[:])
                        else:
                            nc.scalar.copy(o[:, 512 * eh:512 * (eh + 1)], ps[:])
                        if split_dma:
                            # tail mode: ship each half as soon as its copy
                            # lands to shorten the final drain
                            nc.sync.dma_start(
                                y_d.ap()[128 * s:128 * (s + 1),
                                         512 * eh:512 * (eh + 1)],
                                o[:, 512 * eh:512 * (eh + 1)])
                    if not split_dma:
                        # one full-row DMA per s-chunk: halves HWDGE issues
                        nc.sync.dma_start(y_d.ap()[128 * s:128 * (s + 1), :], o[:])

            for P_ in range(2):
                cl, ch = 1024 * P_, 1024 * (P_ + 1)         # column range
                jmax = 8 if P_ == 0 else 16
                for h in range(HG):
                    hk, hp = h // 2, 64 * (h % 2)
                    kts = kt[hp:hp + 64, hk, :]
                    qts = qt[hp:hp + 64, hk, :]
                    pv_tiles = {
                        t: ps_pv.tile([128, 512], F32, tag="pv",
                                      name=f"pv_{h}_{t}")
                        for t in (2 * P_, 2 * P_ + 1)}
                    # (j, t) chunks at 512-col granularity: each sc tile is
                    # [128, 512] (1 PSUM bank) with ONE exp, so the 4-deep sc
                    # ring keeps 4 score/exp chains in flight.  Narrow chunks
                    # consecutive in the same-t j order pack into a shared
                    # tile (right-aligned) to bound the ACT op count; per-t
                    # PV accumulation order stays ascending in j.
                    if P_ == 0:
                        tiles = [[(0, 0)], [(0, 1)], [(1, 0)], [(1, 1)],
                                 [(2, 0), (3, 0)], [(2, 1)], [(3, 1)],
                                 [(4, 1)], [(5, 1)], [(6, 1), (7, 1)]]
                    else:
                        tiles = []
                        for j in range(8):
                            tiles.append([(j, 2)])
                            tiles.append([(j, 3)])
                        tiles += [[(8, 2)], [(8, 3)], [(9, 2)], [(9, 3)],
                                  [(10, 2), (11, 2)], [(10, 3)], [(11, 3)],
                                  [(12, 3)], [(13, 3)], [(14, 3), (15, 3)]]
                    for tl in tiles:
                        dims = [(j, t, max(128 * j, 512 * t)) for j, t in tl]
                        tw = sum(512 * (t + 1) - a0 for j, t, a0 in dims)
                        expt = expt_pool.tile(
                            [128, 512], BF16, tag="expt",
                            name=f"expt_{h}_{P_}_{tl[0][0]}_{tl[0][1]}")
                        sc_ps = ps_sc.tile([128, 512], F32, tag="sc")
                        pos = 512 - tw
                        offs = {}
                        for j, t, a0 in dims:
                            w = 512 * (t + 1) - a0
                            offs[(j, t)] = pos
                            diag = (t == j // 4)    # chunk with the diagonal
                            nc.tensor.matmul(
                                sc_ps[:, pos:pos + w],
                                kts[:, 128 * j:128 * j + 128], qts[:, a0:a0 + w],
                                start=True, stop=not diag)
                            if diag:
                                # causal mask: add -1e30 at cols q' < k of
                                # the 128x128 diagonal tile via I.T @ tneg
                                nc.tensor.matmul(
                                    sc_ps[:, pos:pos + 128],
                                    msk[:, 0, :], msk[:, 1, :], start=False,
                                    stop=True)
                            pos += w
                        # qt/kt carry a 16x scale each -> exp scale /256
                        nc.scalar.activation(
                            expt[:, 512 - tw:], sc_ps[:, 512 - tw:],
                            AF.Exp, scale=INV_SQRT_D / 256.0)
                        # PV accumulation (+ denominator row via 16s column)
                        for j, t, a0 in dims:
                            w = 512 * (t + 1) - a0
                            off = offs[(j, t)]
                            nc.tensor.matmul(
                                pv_tiles[t][:, a0 - 512 * t:a0 - 512 * t + w],
                                vh[:, j, h, :],
                                expt[:, off:off + w],
                                start=(j == 0), stop=(j == 4 * t + 3))
                            if j == 4 * t + 3:
                                # normalize once this sq-chunk completes
                                tsl = slice(512 * t, 512 * (t + 1))
                                rec = dve_tmp.tile([1, 512], F32, tag="rec",
                                                   name=f"rec_{h}_{t}")
                                bc = dve_tmp.tile([128, 512], F32, tag="bc",
                                                  name=f"bc_{h}_{t}")
                                nc.vector.reciprocal(rec[:],
                                                     pv_tiles[t][0:1, :])
                                nc.gpsimd.partition_broadcast(bc[:], rec[:])
                                if h % 2 == 1:
                                    # odd heads: partitions already match
                                    # outt2's upper half — write direct
                                    nc.vector.tensor_tensor(
                                        outt2[64:128, h // 2, tsl],
                                        pv_tiles[t][64:128, :],
                                        bc[64:128, :], ALU.mult)
                                else:
                                    nc.vector.tensor_tensor(
                                        outt[64:128, h // 2, tsl],
                                        pv_tiles[t][64:128, :],
                                        bc[64:128, :], ALU.mult)
                                    # cross-partition stack to rows 0-63
                                    nc.sync.dma_start(
                                        outt2[0:64, h // 2, tsl],
                                        outt[64:128, h // 2, tsl])
                        if P_ == 1 and h == 3 and (11, 2) in offs:
                            # all heads' t=2 columns are final: overlap their
                            # o_proj with the last head's t=3 tail
                            emit_oproj(range(8, 10), ps_a, "mm")

                    if P_ == 0:
                        # PE filler during the exp-bound stretches
                        [lambda: emit_a_strip(2, (0,)),
                         lambda: emit_a_strip(2, (1,)),
                         lambda: emit_a_strip(3, (0,)),
                         lambda: emit_a_strip(3, (1,))][h]()
                    if P_ == 1 and h == 1:
                        emit_oproj(range(0, 8), ps_a, "mm")  # low-priority filler

            # s=10..15 run after the A/B pools close, with a deeper dedicated
            # PSUM pool for a tighter copy/DMA pipeline (s=8..9 were
            # interleaved into head 3's pass-1 stretch above).
            ab_ctx.close()
            with tc.tile_pool(name="ps_c", bufs=6, space="PSUM") as ps_c:
                emit_oproj(range(10, 16), ps_c, "oproj", eng=("act", "dve"))
    nc.compile()
    return nc


_NC_CACHE = {}


def _get_nc():
    if "nc" not in _NC_CACHE:
        _NC_CACHE["nc"] = build_nc()
    return _NC_CACHE["nc"]


def kernel(X, mask, W_qkv, b_qkv, W_o, b_o):
    X = np.asarray(X, dtype=np.float32)
    W_qkv = np.asarray(W_qkv, dtype=np.float32)
    b_qkv = np.asarray(b_qkv, dtype=np.float32)
    W_o = np.asarray(W_o, dtype=np.float32)
    b_o = np.asarray(b_o, dtype=np.float32)

    ident = np.eye(128, dtype=np.float32).astype(BF)
    r = np.arange(128)
    tneg = np.where(r[None, :] < r[:, None], np.float32(-1e30), np.float32(0.0))
    tneg = tneg.astype(BF)                      # tneg[p, n] = -1e30 if n < p

    # fp8 compensation operands (dtype prep for the device kernel):
    # X ~ X8 + Xr8;  W ~ W8 + Wr16/16.  Device computes the uniformly
    # 16x-scaled 16*qkv = (16W8)@X8 + (16W8)@Xr8 + Wr16@X8, so biases ship
    # 16x and the vh ones-column is 16.0.  Pairs pack along a v-dim so each
    # strip/tensor is one DMA issue.
    xv = []
    for b in range(B):
        xT = np.ascontiguousarray(X[b].T)
        x8 = xT.astype(F8)
        xr = (xT - x8.astype(np.float32)).astype(F8)
        # [4, E, 2, 512]: strip-major, (X8, Xr) packed per strip
        xs = np.stack([x8.reshape(E, 4, 512), xr.reshape(E, 4, 512)], axis=2)
        xv.append(np.ascontiguousarray(xs.transpose(1, 0, 2, 3)))
    W8f = W_qkv.astype(F8).astype(np.float32)
    W16 = (16.0 * W8f).astype(F8)              # exact exponent shift
    Wr16 = (16.0 * (W_qkv - W8f)).astype(F8)
    msk = np.stack([ident, tneg], axis=1)      # [128, 2, 128]

    in_maps = []
    for c in range(NC_):
        b, g = c // 4, c % 4
        cols = slice(256 * g, 256 * (g + 1))
        kcols = slice(1024 + 256 * g, 1024 + 256 * (g + 1))
        vcols = slice(2048 + 256 * g, 2048 + 256 * (g + 1))
        wq = np.stack([
            np.concatenate([W16[:, cols], W16[:, kcols]], axis=1),
            np.concatenate([Wr16[:, cols], Wr16[:, kcols]], axis=1)], axis=1)
        wvv = np.stack([W16[:, vcols], Wr16[:, vcols]], axis=1)
        wo = np.ascontiguousarray(
            W_o[256 * g:256 * (g + 1), :].reshape(2, 128, E).transpose(1, 0, 2)).astype(BF)
        bqk = 16.0 * np.concatenate(
            [b_qkv[cols], b_qkv[kcols]]).reshape(4, 128).T.astype(np.float32)
        bv = 16.0 * np.broadcast_to(b_qkv[vcols], (128, 256)).astype(np.float32)
        bias = np.concatenate([bqk, bv], axis=1).astype(np.float32)
        in_maps.append({"xx": xv[b], "wq": np.ascontiguousarray(wq),
                        "wvv": np.ascontiguousarray(wvv), "wo": wo,
                        "msk": np.ascontiguousarray(msk),
                        "bias": np.ascontiguousarray(bias)})

    nc = _get_nc()
    res = run_bass_kernel_spmd(nc, in_maps, core_ids=list(range(NC_)))

    Y = np.zeros((B, S, E), dtype=np.float32)
    for c in range(NC_):
        Y[c // 4] += res.results[c]["y"].astype(np.float32)
    Y += b_o[None, None, :]
    return Y

